# revision 1
# baseline (speedup 1.0000x reference)
"""CapsNet Trainium2 kernel: conv stack + primary caps + dynamic routing.

Distribution: data-parallel convs (batch 256 -> 32/core), then AllToAll to
i-shard (1152 -> 144/core) the routing; one fused AllReduce per routing
iteration carries the unnormalized class sums + softmax denominators.

Layout conventions on device (per core):
  h     [ic_p 128, ic_t 2, b 32, y 20, x 20]   conv1 out / conv2 in
  u     [oc_p 128, oc_t 2, b 32, pos 36]       conv2 out (oc = cap*32+chw)
  a2a blocks: (cap 8, chw_l 4, b 32, pos 36) per destination shard
  routing rows r = (k, i_l) k-major (k=cap, i_l in [0,144))
  class dims f = (c, o) c-major (f = c*16 + o)
"""

import numpy as np
from contextlib import ExitStack

import concourse.bass as bass
import concourse.tile as tile
from concourse import bacc, mybir
from concourse.bass_utils import run_bass_kernel_spmd
from concourse.masks import make_identity

F32 = mybir.dt.float32
F32R = mybir.dt.float32r
AF = mybir.ActivationFunctionType

N_CORES = 8
B = 256
BL = B // N_CORES          # 32 local batch
NCLS = 10
OCH = 16
NI = 1152                  # (k, i_l) rows per core (8*144)
ISH = 144                  # i per core
NITER = 3
NCH = [(0, 512), (512, 1024), (1024, 1152)]


def AP(t_ap, off, dims):
    return bass.AP(tensor=t_ap.tensor, offset=t_ap.offset + off,
                   ap=[list(d) for d in dims])


def build_program():
    nc = bacc.Bacc("TRN2", target_bir_lowering=False, debug=False,
                   num_devices=N_CORES)

    icold = nc.dram_tensor("icold", [81, BL, 400], F32, kind="ExternalInput")
    w1 = nc.dram_tensor("w1", [81, 256], F32, kind="ExternalInput")
    b1 = nc.dram_tensor("b1", [128, 2], F32, kind="ExternalInput")
    w2 = nc.dram_tensor("w2", [81, 256, 256], F32, kind="ExternalInput")
    b2 = nc.dram_tensor("b2", [128, 2], F32, kind="ExternalInput")
    w2s = nc.dram_tensor("w2s", [NI, 160], F32, kind="ExternalInput")
    w3s = nc.dram_tensor("w3s", [160, NI], F32, kind="ExternalInput")
    ssel = nc.dram_tensor("ssel", [160, NCLS], F32, kind="ExternalInput")
    capsum = nc.dram_tensor("capsum", [128, 2, 8], F32, kind="ExternalInput")
    expnd = nc.dram_tensor("expnd", [8, 2, 128], F32, kind="ExternalInput")
    rexpa = nc.dram_tensor("rexpa", [128, 9, 128], F32, kind="ExternalInput")
    rexpb = nc.dram_tensor("rexpb", [16, 9, 128], F32, kind="ExternalInput")
    y = nc.dram_tensor("y", [B, NCLS, OCH], F32, kind="ExternalOutput")

    with tile.TileContext(nc) as tc, ExitStack() as ctx:
        consts = ctx.enter_context(tc.tile_pool(name="consts", bufs=1))
        dram = ctx.enter_context(tc.tile_pool(name="dram", bufs=1, space="DRAM"))

        b1_sb = consts.tile([128, 2], F32)
        nc.sync.dma_start(b1_sb[:], b1[:])
        b2_sb = consts.tile([128, 2], F32)
        nc.sync.dma_start(b2_sb[:], b2[:])
        ident = consts.tile([128, 128], F32)
        make_identity(nc, ident[:])

        a2a_in = dram.tile([N_CORES, BL, 8, 4, 36], F32)   # (m, b, cap, chw_l, pos)
        a2a_out = dram.tile([N_CORES, BL, 8, 4, 36], F32)  # (src, b_l, cap, chw_l, pos)

        with tc.tile_pool(name="hpool", bufs=1) as hpool, \
             tc.tile_pool(name="upool", bufs=1) as upool:
            h = hpool.tile([128, 2, 32, 20, 20], F32R)
            u = upool.tile([128, 2, BL, 36], F32)

            # ============ conv1: 1->256 k9 s1 + ReLU ============
            with tc.tile_pool(name="icolp", bufs=1) as icolp, \
                 tc.tile_pool(name="ps1", bufs=8, space="PSUM") as ps1:
                icol_r = icolp.tile([81, BL, 400], F32R)
                for bc in range(8):
                    ist = icolp.tile([81, 4, 400], F32, tag="ist", bufs=3,
                                     name=f"ist{bc}")
                    nc.sync.dma_start(ist[:], icold[:][:, 4 * bc:4 * bc + 4, :])
                    nc.vector.tensor_copy(icol_r[:, 4 * bc:4 * bc + 4, :], ist[:])
                w1_sb = icolp.tile([81, 256], F32)
                nc.sync.dma_start(w1_sb[:], w1[:])
                w1_r = icolp.tile([81, 256], F32R)
                nc.vector.tensor_copy(w1_r[:], w1_sb[:])
                for t in range(2):
                    for b in range(BL):
                        p = ps1.tile([128, 400], F32, tag="c1")
                        nc.tensor.matmul(
                            p[:],
                            lhsT=w1_r[:, t * 128:(t + 1) * 128],
                            rhs=icol_r[:, b, :],
                            start=True, stop=True)
                        if b % 2 == 0:
                            nc.scalar.activation(
                                h[:, t, b, :, :],
                                p[:].rearrange("p (y x) -> p y x", y=20),
                                AF.Relu, bias=b1_sb[:, t:t + 1], scale=1.0)
                        else:
                            nc.vector.tensor_scalar(
                                out=h[:, t, b, :, :],
                                in0=p[:].rearrange("p (y x) -> p y x", y=20),
                                scalar1=b1_sb[:, t:t + 1], scalar2=0.0,
                                op0=mybir.AluOpType.add,
                                op1=mybir.AluOpType.max)

            # ============ conv2: 256->256 k9 s2 + bias ============
            hv = h[:]
            with tc.tile_pool(name="w2p", bufs=3) as w2p, \
                 tc.tile_pool(name="ps2", bufs=8, space="PSUM") as ps2:
                psum2 = [ps2.tile([128, 8, 36], F32, tag="c2", name=f"c2_{i}")
                         for i in range(8)]
                for kk in range(81):
                    ky, kx = kk // 9, kk % 9
                    w2t = w2p.tile([128, 2, 256], F32)
                    nc.sync.dma_start(
                        w2t[:],
                        AP(w2[:], kk * 65536, [[256, 128], [32768, 2], [1, 256]]))
                    w2r = w2p.tile([128, 2, 256], F32R)
                    nc.vector.tensor_copy(w2r[:], w2t[:])
                    for ic_t in range(2):
                        for oc_t in range(2):
                            lhs = w2r[:, ic_t, oc_t * 128:(oc_t + 1) * 128]
                            for bc in range(4):
                                rhs = hv[:, ic_t, bc * 8:(bc + 1) * 8,
                                         ky:ky + 12:2, kx:kx + 12:2]
                                nc.tensor.matmul(
                                    psum2[oc_t * 4 + bc][:], lhsT=lhs,
                                    rhs=rhs,
                                    start=(kk == 0 and ic_t == 0),
                                    stop=(kk == 80 and ic_t == 1))
                for oc_t in range(2):
                    for bc in range(4):
                        nc.scalar.activation(
                            u[:, oc_t, bc * 8:(bc + 1) * 8, :],
                            psum2[oc_t * 4 + bc][:], AF.Identity,
                            bias=b2_sb[:, oc_t:oc_t + 1], scale=1.0)

            # ============ squash over i per (b, cap) ============
            with tc.tile_pool(name="sqp", bufs=1) as sqp, \
                 tc.tile_pool(name="ps3", bufs=2, space="PSUM") as ps3:
                capsum_sb = sqp.tile([128, 2, 8], F32)
                nc.sync.dma_start(capsum_sb[:], capsum[:])
                expnd_sb = sqp.tile([8, 2, 128], F32)
                nc.sync.dma_start(expnd_sb[:], expnd[:])

                usq = sqp.tile([128, 2, BL * 36], F32)
                uv2 = u[:].rearrange("p t b q -> p t (b q)")
                pnorm = ps3.tile([8, BL * 36], F32, tag="pn")
                for oc_t in range(2):
                    nc.vector.tensor_mul(usq[:, oc_t, :], uv2[:, oc_t, :],
                                         uv2[:, oc_t, :])
                    for (n0, n1) in NCH:
                        nc.tensor.matmul(
                            pnorm[:, n0:n1],
                            lhsT=capsum_sb[:, oc_t, :],
                            rhs=usq[:, oc_t, n0:n1],
                            start=(oc_t == 0), stop=(oc_t == 1))
                normsq = sqp.tile([8, BL], F32)
                nc.vector.reduce_sum(
                    out=normsq[:],
                    in_=pnorm[:].rearrange("c (b q) -> c b q", q=36),
                    axis=mybir.AxisListType.X)
                scl = sqp.tile([8, BL], F32)
                nc.scalar.sqrt(scl[:], normsq[:])
                nc.vector.tensor_scalar_add(scl[:], scl[:], 1e-10)
                onep = sqp.tile([8, BL], F32)
                nc.vector.tensor_scalar_add(onep[:], normsq[:], 1.0)
                den = sqp.tile([8, BL], F32)
                nc.vector.tensor_mul(den[:], scl[:], onep[:])
                rden = sqp.tile([8, BL], F32)
                nc.vector.reciprocal(rden[:], den[:])
                fac = sqp.tile([8, BL], F32)
                nc.vector.tensor_mul(fac[:], normsq[:], rden[:])
                sfac = sqp.tile([128, 2, BL], F32)
                for oc_t in range(2):
                    pfac = ps3.tile([128, BL], F32, tag="pf")
                    nc.tensor.matmul(pfac[:], lhsT=expnd_sb[:, oc_t, :],
                                     rhs=fac[:], start=True, stop=True)
                    nc.scalar.copy(sfac[:, oc_t, :], pfac[:])
                    nc.vector.tensor_mul(
                        u[:, oc_t, :, :], u[:, oc_t, :, :],
                        AP(sfac[:], oc_t * BL, [[2 * BL, 128], [1, BL], [0, 36]]))

            # ============ AllToAll pack: blocks (b, cap, chw_l, pos) ============
            # oc channels are host-permuted: partition q = (m%4)*32 + cap*4 + chw_l
            for m in range(N_CORES):
                nc.sync.dma_start(
                    AP(a2a_in[:], m * BL * 1152,
                       [[36, 32], [1152, BL], [1, 36]]),
                    u[(m % 4) * 32:(m % 4) * 32 + 32, m // 4, :, :])
        nc.gpsimd.collective_compute(
            "AllToAll", mybir.AluOpType.bypass,
            replica_groups=[list(range(N_CORES))],
            ins=[a2a_in.opt()], outs=[a2a_out.opt()])

        # ============ routing setup ============
        rts = ctx.enter_context(tc.tile_pool(name="rts", bufs=1))
        u3ki = rts.tile([128, 2, 8, 4, 36], F32)   # (b_p, bh, cap, chw_l, pos)
        u2ki = rts.tile([128, 9, 256], F32)        # ((k,i) rows, tile, b)
        w2s_sb = rts.tile([128, 9, 160], F32)
        w3sA = rts.tile([128, NI], F32)
        w3sB = rts.tile([32, NI], F32)
        sselA = rts.tile([128, NCLS], F32)
        sselB = rts.tile([32, NCLS], F32)
        rexpa_sb = rts.tile([128, 9, 128], F32)
        rexpb_sb = rts.tile([16, 9, 128], F32)
        e2A = rts.tile([128, NI], F32R)
        e2B = rts.tile([32, NI], F32R)
        u3r = rts.tile([128, 2, 8, 4, 36], F32R)
        o3r = rts.tile([128, 2, 160], F32R)
        sselAr = rts.tile([128, NCLS], F32R)
        sselBr = rts.tile([32, NCLS], F32R)
        b_sb = rts.tile([NCLS, ISH], F32)
        expb = rts.tile([NCLS, ISH], F32)
        zloc = rts.tile([NCLS, 1], F32)
        expT = rts.tile([128, NCLS], F32)
        expT2 = rts.tile([16, NCLS], F32)
        e8 = rts.tile([128, 9, NCLS], F32)
        ew2 = rts.tile([128, 9, 160], F32)
        stg = rts.tile([128, 2, 160], F32)
        sg = rts.tile([128, 2, 160], F32)
        zrow = rts.tile([128, NCLS], F32)
        rzrow = rts.tile([128, NCLS], F32)
        sqs = rts.tile([128, 2, 160], F32)
        nrm = rts.tile([128, 2, OCH], F32)
        o3 = rts.tile([128, 2, 160], F32)
        uvf = rts.tile([NCLS, ISH], F32)

        nc.sync.dma_start(
            w2s_sb[:], AP(w2s[:], 0, [[160, 128], [128 * 160, 9], [1, 160]]))
        nc.sync.dma_start(w3sA[:], w3s[:][0:128, :])
        nc.sync.dma_start(w3sB[:], w3s[:][128:160, :])
        nc.sync.dma_start(sselA[:], ssel[:][0:128, :])
        nc.sync.dma_start(sselB[:], ssel[:][128:160, :])
        nc.sync.dma_start(
            rexpa_sb[:], AP(rexpa[:], 0, [[9 * 128, 128], [128, 9], [1, 128]]))
        nc.sync.dma_start(
            rexpb_sb[:], AP(rexpb[:], 0, [[9 * 128, 16], [128, 9], [1, 128]]))
        nc.vector.memset(b_sb[:], 0.0)

        # u3ki receive: partition (src, b_l) dense; free (cap, chw_l, pos) dense
        for bh in range(2):
            nc.sync.dma_start(
                u3ki[:, bh, :, :, :],
                AP(a2a_out[:], bh * 128 * 1152, [[1152, 128], [1, 1152]]))
        nc.vector.tensor_copy(u3r[:], u3ki[:])
        nc.vector.tensor_copy(sselAr[:], sselA[:])
        nc.vector.tensor_copy(sselBr[:], sselB[:])
        # transposes -> u2ki rows (k,i)
        with tc.tile_pool(name="psT", bufs=4, space="PSUM") as psT:
            u3v = u3ki[:].rearrange("p h c w q -> p h (c w q)")
            for bh in range(2):
                for t in range(9):
                    pt = psT.tile([128, 128], F32, tag="tr")
                    nc.tensor.transpose(pt[:], u3v[:, bh, t * 128:(t + 1) * 128],
                                        ident[:])
                    nc.scalar.copy(u2ki[:, t, bh * 128:(bh + 1) * 128], pt[:])

        # ============ routing iterations ============
        ar_in = dram.tile([2, 129, 160], F32)
        ar_out = dram.tile([2, 129, 160], F32)
        zz = rts.tile([2, 160], F32)
        nc.vector.memset(zz[:], 0.0)
        for bh in range(2):
            nc.sync.dma_start(ar_in[bh, 128, :], zz[bh:bh + 1, :])
        psS = ctx.enter_context(tc.tile_pool(name="psS", bufs=2, space="PSUM"))
        psM = ctx.enter_context(tc.tile_pool(name="psM", bufs=2, space="PSUM"))
        psU = ctx.enter_context(tc.tile_pool(name="psU", bufs=1, space="PSUM"))

        for it in range(NITER):
            if it == 0:
                # b == 0: exp(b) = 1 everywhere, Z_local = 144
                nc.vector.memset(zloc[:], float(ISH))
            else:
                nc.scalar.activation(expb[:], b_sb[:], AF.Exp)
                nc.vector.reduce_sum(out=zloc[:], in_=expb[:],
                                     axis=mybir.AxisListType.X)
                pt1 = psS.tile([128, NCLS], F32, tag="sm", bufs=1)
                nc.tensor.transpose(pt1[:], expb[:, 0:128], ident[0:NCLS, 0:NCLS])
                nc.scalar.copy(expT[:], pt1[:])
                pt2 = psS.tile([16, NCLS], F32, tag="sm", bufs=1)
                nc.tensor.transpose(pt2[:], expb[:, 128:144], ident[0:NCLS, 0:NCLS])
                nc.scalar.copy(expT2[:], pt2[:])
                pe8 = psS.tile([128, 9, NCLS], F32, tag="e8g", bufs=1)
                for t in range(9):
                    nc.tensor.matmul(pe8[:, t, :], lhsT=rexpa_sb[:, t, :],
                                     rhs=expT[:], start=True, stop=False)
                    nc.tensor.matmul(pe8[:, t, :], lhsT=rexpb_sb[:, t, :],
                                     rhs=expT2[:], start=False, stop=True)
                nc.vector.tensor_mul(
                    ew2[:].rearrange("p t (c o) -> p t c o", c=NCLS),
                    w2s_sb[:].rearrange("p t (c o) -> p t c o", c=NCLS),
                    AP(pe8[:], 0, [[90, 128], [10, 9], [1, 10], [0, 16]]))
            rhs_w = w2s_sb if it == 0 else ew2
            for bh in range(2):
                pst = psM.tile([128, 160], F32, tag="st", bufs=1)
                for t in range(9):
                    nc.tensor.matmul(pst[:],
                                     lhsT=u2ki[:, t, bh * 128:(bh + 1) * 128],
                                     rhs=rhs_w[:, t, :], start=(t == 0), stop=(t == 8))
                nc.scalar.copy(stg[:, bh, :], pst[:])
                nc.sync.dma_start(ar_in[bh, 0:128, :], stg[:, bh, :])
            nc.sync.dma_start(ar_in[0, 128, 0:NCLS], zloc[:])
            nc.gpsimd.collective_compute(
                "AllReduce", mybir.AluOpType.add,
                replica_groups=[list(range(N_CORES))],
                ins=[ar_in.opt()], outs=[ar_out.opt()])
            for bh in range(2):
                nc.sync.dma_start(sg[:, bh, :], ar_out[bh, 0:128, :])
            nc.sync.dma_start(
                zrow[:], AP(ar_out[:], 128 * 160, [[0, 128], [1, NCLS]]))
            nc.vector.reciprocal(rzrow[:], zrow[:])
            nc.vector.tensor_mul(
                sg[:].rearrange("p t (c o) -> p t c o", c=NCLS),
                sg[:].rearrange("p t (c o) -> p t c o", c=NCLS),
                AP(rzrow[:], 0, [[NCLS, 128], [0, 2], [1, 10], [0, 16]]))
            nc.vector.tensor_mul(sqs[:], sg[:].rearrange("p t f -> p (t f)"),
                                 sg[:].rearrange("p t f -> p (t f)"))
            nc.vector.reduce_sum(
                out=nrm[:],
                in_=sqs[:].rearrange("p t (c o) -> p t o c", c=NCLS),
                axis=mybir.AxisListType.X)
            scl2 = rts.tile([128, 2, OCH], F32)
            nc.scalar.sqrt(scl2[:], nrm[:])
            nc.vector.tensor_scalar_add(scl2[:], scl2[:], 1e-10)
            onep2 = rts.tile([128, 2, OCH], F32)
            nc.vector.tensor_scalar_add(onep2[:], nrm[:], 1.0)
            den2 = rts.tile([128, 2, OCH], F32)
            nc.vector.tensor_mul(den2[:], scl2[:], onep2[:])
            rden2 = rts.tile([128, 2, OCH], F32)
            nc.vector.reciprocal(rden2[:], den2[:])
            fac2 = rts.tile([128, 2, OCH], F32)
            nc.vector.tensor_mul(fac2[:], nrm[:], rden2[:])
            nc.vector.tensor_mul(
                o3[:].rearrange("p t (c o) -> p t c o", c=NCLS),
                sg[:].rearrange("p t (c o) -> p t c o", c=NCLS),
                AP(fac2[:], 0, [[2 * OCH, 128], [OCH, 2], [0, 10], [1, OCH]]))

            if it == NITER - 1:
                for bh in range(2):
                    nc.sync.dma_start(
                        AP(y[:], bh * 128 * 160, [[160, 128], [1, 160]]),
                        o3[:, bh, :])
            else:
                nc.vector.tensor_copy(o3r[:], o3[:])
                u3f = u3r[:].rearrange("p h c w q -> p h (c w q)")
                for (n0, n1) in NCH:
                    for mt, msz in ((0, 128), (1, 32)):
                        pA = psM.tile([128, 512], F32, tag="pA")
                        for bh in range(2):
                            nc.tensor.matmul(
                                pA[:msz, 0:n1 - n0],
                                lhsT=o3r[:, bh, mt * 128:mt * 128 + msz],
                                rhs=u3f[:, bh, n0:n1],
                                start=(bh == 0), stop=(bh == 1))
                        e2dst, w3sx = ((e2A, w3sA) if mt == 0 else (e2B, w3sB))
                        nc.vector.tensor_mul(e2dst[:, n0:n1], pA[:msz, 0:n1 - n0],
                                             w3sx[:, n0:n1])
                puv = psU.tile([NCLS, NI], F32, tag="uv")
                for kc, (ssb, e2b) in enumerate(((sselAr, e2A), (sselBr, e2B))):
                    for (n0, n1) in NCH:
                        nc.tensor.matmul(
                            puv[:, n0:n1], lhsT=ssb[:],
                            rhs=e2b[:, n0:n1],
                            start=(kc == 0), stop=(kc == 1))
                puvv = puv[:].rearrange("c (k i) -> c k i", k=8)
                nc.scalar.copy(uvf[:], puvv[:, 0, :])
                for k in range(1, 8):
                    nc.vector.tensor_add(uvf[:], uvf[:], puvv[:, k, :])
                nc.vector.tensor_add(b_sb[:], b_sb[:], uvf[:])

    nc.compile()
    return nc


_CACHE = {}


def _get_program():
    if "nc" not in _CACHE:
        _CACHE["nc"] = build_program()
    return _CACHE["nc"]


def _host_inputs(x, conv_w, conv_b, prim_w, prim_b, digit_w):
    x = np.asarray(x, dtype=np.float32)
    conv_w = np.asarray(conv_w, dtype=np.float32)
    conv_b = np.asarray(conv_b, dtype=np.float32)
    prim_w = np.asarray(prim_w, dtype=np.float32)
    prim_b = np.asarray(prim_b, dtype=np.float32)
    digit_w = np.asarray(digit_w, dtype=np.float32)

    # im2col of x: (B, 1, 28, 28) -> (B, 81, 400) windows
    xi = x.reshape(B, 28, 28)
    s0, s1, s2 = xi.strides
    win = np.lib.stride_tricks.as_strided(
        xi, shape=(B, 9, 9, 20, 20), strides=(s0, s1, s2, s1, s2))
    icold_full = np.ascontiguousarray(
        win.reshape(B, 81, 400).transpose(1, 0, 2))      # (81, B, 400)

    w1 = np.ascontiguousarray(conv_w.reshape(256, 81).T)
    b1 = np.ascontiguousarray(conv_b.reshape(2, 128).T)
    # permute conv2 output channels: oc=(cap*32+chw) -> q_global so that the
    # AllToAll pack per shard m reads 32 contiguous partitions
    oc = np.arange(256)
    cap_, chw = oc // 32, oc % 32
    qg = (chw // 16) * 128 + (chw // 4 % 4) * 32 + cap_ * 4 + (chw % 4)
    perm_inv = np.argsort(qg)
    w2 = np.ascontiguousarray(
        prim_w.reshape(256, 256, 81).transpose(2, 1, 0)[:, :, perm_inv])
    b2 = np.ascontiguousarray(prim_b[perm_inv].reshape(2, 128).T)
    ssel = np.zeros((160, NCLS), np.float32)
    for c in range(NCLS):
        for o in range(OCH):
            ssel[c * OCH + o, c] = 1.0 / B
    capsum = np.zeros((128, 2, 8), np.float32)
    expnd = np.zeros((8, 2, 128), np.float32)
    for p in range(128):
        for oc_t in range(2):
            c_of_p = (p % 32) // 4
            capsum[p, oc_t, c_of_p] = 1.0
            expnd[c_of_p, oc_t, p] = 1.0
    rexpa = np.zeros((128, 9, 128), np.float32)
    rexpb = np.zeros((16, 9, 128), np.float32)
    for t in range(9):
        for p in range(128):
            i_l = (t * 128 + p) % ISH
            if i_l < 128:
                rexpa[i_l, t, p] = 1.0
            else:
                rexpb[i_l - 128, t, p] = 1.0

    in_maps = []
    for m in range(N_CORES):
        dw = digit_w[m * ISH:(m + 1) * ISH]              # (144, 10, 16, 8)
        w2s_h = np.ascontiguousarray(dw.transpose(3, 0, 1, 2).reshape(NI, 160))
        w3s_h = np.ascontiguousarray(dw.transpose(1, 2, 3, 0).reshape(160, NI))
        in_maps.append({
            "icold": np.ascontiguousarray(icold_full[:, m * BL:(m + 1) * BL, :]),
            "w1": w1, "b1": b1, "w2": w2, "b2": b2,
            "w2s": w2s_h, "w3s": w3s_h, "ssel": ssel,
            "capsum": capsum, "expnd": expnd, "rexpa": rexpa, "rexpb": rexpb,
        })
    return in_maps


def kernel(x, conv_w, conv_b, prim_w, prim_b, digit_w, trace=False):
    nc = _get_program()
    in_maps = _host_inputs(x, conv_w, conv_b, prim_w, prim_b, digit_w)
    res = run_bass_kernel_spmd(nc, in_maps, list(range(N_CORES)), trace=trace)
    out = res.results[0]["y"].reshape(B, NCLS, OCH, 1).astype(np.float32)
    if trace:
        return out, res
    return out



# revision 9
# speedup vs baseline: 1.2254x; 1.2254x over previous
"""CapsNet Trainium2 kernel: conv stack + primary caps + dynamic routing.

Distribution: data-parallel convs (batch 256 -> 32/core), then AllToAll to
i-shard (1152 -> 144/core) the routing; one fused AllReduce per routing
iteration carries the unnormalized class sums + softmax denominators. The
final iteration uses a ReduceScatter instead (each core keeps only its own
batch block of the output) and y is gathered on the host.

All matmul operands, DMA payloads, and collective payloads are bf16
(fp32 psum accumulation); squash sqrt is computed as exp(0.5*ln(n)-ln(1+n))
so the Activation engine needs only the natural_log_exp_and_others table.

Layout conventions on device (per core):
  h     [ic_p 128, ic_t 2, b 32, y 20, x 20]   conv1 out / conv2 in (bf16)
  u     [oc_p 128, oc_t 2, b 32, pos 36]       conv2 out (oc = cap*32+chw)
  a2a blocks: (cap 8, chw_l 4, b 32, pos 36) per destination shard
  routing rows r = (k, i_l) k-major (k=cap, i_l in [0,144))
  class dims f = (c, o) c-major (f = c*16 + o)
"""

import numpy as np
from contextlib import ExitStack

import concourse.bass as bass
import concourse.tile as tile
from concourse import bacc, mybir
from concourse.bass_utils import run_bass_kernel_spmd
from concourse.masks import make_identity

F32 = mybir.dt.float32
BF16 = mybir.dt.bfloat16
AF = mybir.ActivationFunctionType
ALU = mybir.AluOpType

N_CORES = 8
B = 256
BL = B // N_CORES          # 32 local batch
NCLS = 10
OCH = 16
NI = 1152                  # (k, i_l) rows per core (8*144)
ISH = 144                  # i per core
NITER = 3
NCH = [(0, 512), (512, 1024), (1024, 1152)]
ACT_SET_LN_EXP = 6         # natural_log_exp_and_others in act_info.json


def AP(t_ap, off, dims):
    return bass.AP(tensor=t_ap.tensor, offset=t_ap.offset + off,
                   ap=[list(d) for d in dims])


def build_program():
    nc = bacc.Bacc("TRN2", target_bir_lowering=False, debug=False,
                   num_devices=N_CORES)

    icold = nc.dram_tensor("icold", [81, BL, 400], BF16, kind="ExternalInput")
    w1 = nc.dram_tensor("w1", [81, 256], BF16, kind="ExternalInput")
    b1 = nc.dram_tensor("b1", [128, 2], F32, kind="ExternalInput")
    w2 = nc.dram_tensor("w2", [81, 256, 256], BF16, kind="ExternalInput")
    b2 = nc.dram_tensor("b2", [128, 2], F32, kind="ExternalInput")
    w2s = nc.dram_tensor("w2s", [NI, 160], BF16, kind="ExternalInput")
    w3s = nc.dram_tensor("w3s", [160, NI], BF16, kind="ExternalInput")
    ssel = nc.dram_tensor("ssel", [160, NCLS], BF16, kind="ExternalInput")
    capsum = nc.dram_tensor("capsum", [128, 2, 8], BF16, kind="ExternalInput")
    expnd = nc.dram_tensor("expnd", [8, 2, 128], BF16, kind="ExternalInput")
    rexpa = nc.dram_tensor("rexpa", [128, 9, 128], BF16, kind="ExternalInput")
    rexpb = nc.dram_tensor("rexpb", [16, 9, 128], BF16, kind="ExternalInput")
    y = nc.dram_tensor("y", [BL, NCLS * OCH], F32, kind="ExternalOutput")

    with tile.TileContext(nc) as tc, ExitStack() as ctx:
        # pin the act table once: exp + ln + relu + identity/copy all live in
        # natural_log_exp_and_others, so no per-iteration table switches
        ld = mybir.InstLoadActFuncSet(
            name=nc.get_next_instruction_name(), ins=[], outs=[])
        ld.act_func_set_id = ACT_SET_LN_EXP
        nc.scalar.add_instruction(ld)

        consts = ctx.enter_context(tc.tile_pool(name="consts", bufs=1))
        dram = ctx.enter_context(tc.tile_pool(name="dram", bufs=1, space="DRAM"))

        b1_sb = consts.tile([128, 2], F32)
        nc.sync.dma_start(b1_sb[:], b1[:])
        b2_sb = consts.tile([128, 2], F32)
        nc.sync.dma_start(b2_sb[:], b2[:])
        ident = consts.tile([128, 128], BF16)
        make_identity(nc, ident[:])
        ones_a = consts.tile([128, 8], BF16)
        nc.vector.memset(ones_a[:], 1.0)
        ones_b = consts.tile([16, 8], BF16)
        nc.vector.memset(ones_b[:], 1.0)

        a2a_in = dram.tile([N_CORES, BL, 8, 4, 36], BF16)  # (m, b, cap, chw_l, pos)
        a2a_out = dram.tile([N_CORES, BL, 8, 4, 36], BF16)  # (src, b_l, cap, chw_l, pos)

        with tc.tile_pool(name="hpool", bufs=1) as hpool, \
             tc.tile_pool(name="upool", bufs=1) as upool:
            h = hpool.tile([128, 2, 32, 20, 20], BF16)
            u = upool.tile([128, 2, BL, 36], BF16)

            # ============ conv1: 1->256 k9 s1 + ReLU ============
            with tc.tile_pool(name="icolp", bufs=1) as icolp, \
                 tc.tile_pool(name="ps1", bufs=8, space="PSUM") as ps1:
                icol = icolp.tile([81, BL, 400], BF16)
                nc.sync.dma_start(icol[:], icold[:])
                w1_sb = icolp.tile([81, 256], BF16)
                nc.sync.dma_start(w1_sb[:], w1[:])
                for t in range(2):
                    for b in range(BL):
                        p = ps1.tile([128, 400], F32, tag="c1")
                        nc.tensor.matmul(
                            p[:],
                            lhsT=w1_sb[:, t * 128:(t + 1) * 128],
                            rhs=icol[:, b, :],
                            start=True, stop=True)
                        pr = p[:].rearrange("p (y x) -> p y x", y=20)
                        if b % 2 == 0:
                            nc.scalar.activation(
                                h[:, t, b, :, :], pr, AF.Relu,
                                bias=b1_sb[:, t:t + 1], scale=1.0)
                        else:
                            nc.vector.tensor_scalar(
                                out=h[:, t, b, :, :], in0=pr,
                                scalar1=b1_sb[:, t:t + 1], scalar2=0.0,
                                op0=ALU.add, op1=ALU.max)

            # ============ conv2: 256->256 k9 s2 + bias ============
            hv = h[:]
            with tc.tile_pool(name="w2p", bufs=3) as w2p, \
                 tc.tile_pool(name="ps2", bufs=8, space="PSUM") as ps2:
                psum2 = [ps2.tile([128, 8, 36], F32, tag="c2", name=f"c2_{i}")
                         for i in range(8)]
                for kk in range(81):
                    ky, kx = kk // 9, kk % 9
                    w2t = w2p.tile([128, 2, 256], BF16)
                    nc.sync.dma_start(
                        w2t[:],
                        AP(w2[:], kk * 65536, [[256, 128], [32768, 2], [1, 256]]))
                    for ic_t in range(2):
                        for oc_t in range(2):
                            lhs = w2t[:, ic_t, oc_t * 128:(oc_t + 1) * 128]
                            for bc in range(4):
                                rhs = hv[:, ic_t, bc * 8:(bc + 1) * 8,
                                         ky:ky + 12:2, kx:kx + 12:2]
                                nc.tensor.matmul(
                                    psum2[oc_t * 4 + bc][:], lhsT=lhs,
                                    rhs=rhs,
                                    start=(kk == 0 and ic_t == 0),
                                    stop=(kk == 80 and ic_t == 1))
                for oc_t in range(2):
                    for bc in range(4):
                        i8 = oc_t * 4 + bc
                        dst = u[:, oc_t, bc * 8:(bc + 1) * 8, :]
                        if i8 % 2 == 0:
                            nc.scalar.activation(
                                dst, psum2[i8][:], AF.Identity,
                                bias=b2_sb[:, oc_t:oc_t + 1], scale=1.0)
                        else:
                            nc.vector.tensor_scalar(
                                out=dst, in0=psum2[i8][:],
                                scalar1=b2_sb[:, oc_t:oc_t + 1], scalar2=None,
                                op0=ALU.add)

            # ============ squash over i per (b, cap) ============
            with tc.tile_pool(name="sqp", bufs=1) as sqp, \
                 tc.tile_pool(name="ps3", bufs=2, space="PSUM") as ps3:
                capsum_sb = sqp.tile([128, 2, 8], BF16)
                nc.sync.dma_start(capsum_sb[:], capsum[:])
                expnd_sb = sqp.tile([8, 2, 128], BF16)
                nc.sync.dma_start(expnd_sb[:], expnd[:])

                usq = sqp.tile([128, 2, BL * 36], BF16)
                uv2 = u[:].rearrange("p t b q -> p t (b q)")
                pnorm = ps3.tile([8, BL * 36], F32, tag="pn")
                for oc_t in range(2):
                    nc.vector.tensor_mul(usq[:, oc_t, :], uv2[:, oc_t, :],
                                         uv2[:, oc_t, :])
                    for (n0, n1) in NCH:
                        nc.tensor.matmul(
                            pnorm[:, n0:n1],
                            lhsT=capsum_sb[:, oc_t, :],
                            rhs=usq[:, oc_t, n0:n1],
                            start=(oc_t == 0), stop=(oc_t == 1))
                normsq = sqp.tile([8, BL], F32)
                nc.vector.reduce_sum(
                    out=normsq[:],
                    in_=pnorm[:].rearrange("c (b q) -> c b q", q=36),
                    axis=mybir.AxisListType.X)
                # fac = sqrt(n)/(1+n) = exp(0.5*ln(n) - ln(1+n))
                lg1 = sqp.tile([8, BL], F32)
                nc.scalar.activation(lg1[:], normsq[:], AF.Ln)
                lg2 = sqp.tile([8, BL], F32)
                nc.scalar.activation(lg2[:], normsq[:], AF.Ln, bias=1.0,
                                     scale=1.0)
                tt = sqp.tile([8, BL], F32)
                nc.vector.scalar_tensor_tensor(
                    out=tt[:], in0=lg1[:], scalar=0.5, in1=lg2[:],
                    op0=ALU.mult, op1=ALU.subtract)
                facb = sqp.tile([8, BL], BF16)
                nc.scalar.activation(facb[:], tt[:], AF.Exp)
                sfac = sqp.tile([128, 2, BL], BF16)
                for oc_t in range(2):
                    pfac = ps3.tile([128, BL], F32, tag="pf")
                    nc.tensor.matmul(pfac[:], lhsT=expnd_sb[:, oc_t, :],
                                     rhs=facb[:], start=True, stop=True)
                    nc.scalar.copy(sfac[:, oc_t, :], pfac[:])
                    eng = nc.vector if oc_t == 0 else nc.gpsimd
                    eng.tensor_mul(
                        u[:, oc_t, :, :], u[:, oc_t, :, :],
                        AP(sfac[:], oc_t * BL, [[2 * BL, 128], [1, BL], [0, 36]]))

            # ============ AllToAll pack: blocks (b, cap, chw_l, pos) ============
            # oc channels are host-permuted: partition q = (m%4)*32 + cap*4 + chw_l
            for m in range(N_CORES):
                nc.sync.dma_start(
                    AP(a2a_in[:], m * BL * 1152,
                       [[36, 32], [1152, BL], [1, 36]]),
                    u[(m % 4) * 32:(m % 4) * 32 + 32, m // 4, :, :])
        nc.gpsimd.collective_compute(
            "AllToAll", ALU.bypass,
            replica_groups=[list(range(N_CORES))],
            ins=[a2a_in.opt()], outs=[a2a_out.opt()])

        # ============ routing setup ============
        rts = ctx.enter_context(tc.tile_pool(name="rts", bufs=1))
        u3ki = rts.tile([128, 2, 8, 4, 36], BF16)  # (b_p, bh, cap, chw_l, pos)
        u2ki = rts.tile([128, 9, 256], BF16)       # ((k,i) rows, tile, b)
        w2s_sb = rts.tile([128, 9, 160], BF16)
        w3sA = rts.tile([128, NI], BF16)
        w3sB = rts.tile([32, NI], BF16)
        sselA = rts.tile([128, NCLS], BF16)
        sselB = rts.tile([32, NCLS], BF16)
        rexpa_sb = rts.tile([128, 9, 128], BF16)
        rexpb_sb = rts.tile([16, 9, 128], BF16)
        e2A = rts.tile([128, NI], BF16)
        e2B = rts.tile([32, NI], BF16)
        b_sb = rts.tile([NCLS, ISH], F32)
        expb = rts.tile([NCLS, ISH], BF16)
        zloc = rts.tile([NCLS, 1], F32)
        zlocb = rts.tile([NCLS, 1], BF16)
        expT = rts.tile([128, NCLS], BF16)
        expT2 = rts.tile([16, NCLS], BF16)
        ew2 = rts.tile([128, 9, 160], BF16)
        stgsb = rts.tile([128, 2, 160], BF16)
        sg = rts.tile([128, 2, 160], BF16)
        sgf = rts.tile([128, 2, 160], F32)
        zrow = rts.tile([128, NCLS], BF16)
        rzrow = rts.tile([128, NCLS], F32)
        sqs = rts.tile([128, 2, 160], F32)
        nrm = rts.tile([128, 2, OCH], F32)
        o3 = rts.tile([128, 2, 160], BF16)
        uvf = rts.tile([NCLS, ISH], F32)

        nc.sync.dma_start(
            w2s_sb[:], AP(w2s[:], 0, [[160, 128], [128 * 160, 9], [1, 160]]))
        nc.sync.dma_start(w3sA[:], w3s[:][0:128, :])
        nc.sync.dma_start(w3sB[:], w3s[:][128:160, :])
        nc.sync.dma_start(sselA[:], ssel[:][0:128, :])
        nc.sync.dma_start(sselB[:], ssel[:][128:160, :])
        nc.sync.dma_start(
            rexpa_sb[:], AP(rexpa[:], 0, [[9 * 128, 128], [128, 9], [1, 128]]))
        nc.sync.dma_start(
            rexpb_sb[:], AP(rexpb[:], 0, [[9 * 128, 16], [128, 9], [1, 128]]))
        nc.vector.memset(b_sb[:], 0.0)

        # u3ki receive: partition (src, b_l) dense; free (cap, chw_l, pos) dense
        for bh in range(2):
            nc.sync.dma_start(
                u3ki[:, bh, :, :, :],
                AP(a2a_out[:], bh * 128 * 1152, [[1152, 128], [1, 1152]]))
        u3f = u3ki[:].rearrange("p h c w q -> p h (c w q)")
        # transposes -> u2ki rows (k,i)
        with tc.tile_pool(name="psT", bufs=4, space="PSUM") as psT:
            for bh in range(2):
                for t in range(9):
                    pt = psT.tile([128, 128], BF16, tag="tr")
                    nc.tensor.transpose(pt[:], u3f[:, bh, t * 128:(t + 1) * 128],
                                        ident[:])
                    i18 = bh * 9 + t
                    dst = u2ki[:, t, bh * 128:(bh + 1) * 128]
                    if i18 % 2 == 0:
                        nc.scalar.copy(dst, pt[:])
                    else:
                        nc.vector.tensor_copy(dst, pt[:])

        # ============ routing iterations ============
        ar_in = dram.tile([2, 129, 160], BF16)
        ar_out = dram.tile([2, 129, 160], BF16)
        rs_in = dram.tile([N_CORES, BL + 1, 160], BF16)
        rs_out = dram.tile([BL + 1, 160], BF16)
        zz = rts.tile([2, 160], BF16)
        nc.vector.memset(zz[:], 0.0)
        nc.sync.dma_start(
            AP(ar_in[:], 128 * 160, [[129 * 160, 2], [1, 160]]), zz[:])
        psS = ctx.enter_context(tc.tile_pool(name="psS", bufs=2, space="PSUM"))
        psM = ctx.enter_context(tc.tile_pool(name="psM", bufs=2, space="PSUM"))
        psU = ctx.enter_context(tc.tile_pool(name="psU", bufs=1, space="PSUM"))

        for it in range(NITER):
            if it == 0:
                # b == 0: exp(b) = 1 everywhere, Z_local = 144
                nc.vector.memset(zlocb[:], float(ISH))
            else:
                nc.scalar.activation(expb[:], b_sb[:], AF.Exp)
                nc.vector.reduce_sum(out=zloc[:], in_=expb[:],
                                     axis=mybir.AxisListType.X)
                if it < NITER - 1:
                    nc.vector.tensor_copy(zlocb[:], zloc[:])
                pt1 = psS.tile([128, NCLS], BF16, tag="sm", bufs=1)
                nc.tensor.transpose(pt1[:], expb[:, 0:128], ident[0:NCLS, 0:NCLS])
                nc.scalar.copy(expT[:], pt1[:])
                pt2 = psS.tile([16, NCLS], BF16, tag="sm", bufs=1)
                nc.tensor.transpose(pt2[:], expb[:, 128:144], ident[0:NCLS, 0:NCLS])
                nc.scalar.copy(expT2[:], pt2[:])
                pe8 = psS.tile([128, 9, NCLS], F32, tag="e8g", bufs=1)
                for t in range(9):
                    nc.tensor.matmul(pe8[:, t, :], lhsT=rexpa_sb[:, t, :],
                                     rhs=expT[:], start=True, stop=False)
                    nc.tensor.matmul(pe8[:, t, :], lhsT=rexpb_sb[:, t, :],
                                     rhs=expT2[:], start=False, stop=True)
                nc.vector.tensor_mul(
                    ew2[:].rearrange("p t (c o) -> p t c o", c=NCLS),
                    w2s_sb[:].rearrange("p t (c o) -> p t c o", c=NCLS),
                    AP(pe8[:], 0, [[90, 128], [10, 9], [1, 10], [0, 16]]))
            rhs_w = w2s_sb if it == 0 else ew2
            for bh in range(2):
                pst = psM.tile([128, 160], F32, tag="st", bufs=1)
                for t in range(9):
                    nc.tensor.matmul(pst[:],
                                     lhsT=u2ki[:, t, bh * 128:(bh + 1) * 128],
                                     rhs=rhs_w[:, t, :], start=(t == 0), stop=(t == 8))
                if bh == 0:
                    nc.scalar.copy(stgsb[:, bh, :], pst[:])
                else:
                    nc.vector.tensor_copy(stgsb[:, bh, :], pst[:])

            if it < NITER - 1:
                # ---- AllReduce of [s_j partials | Z partials] ----
                nc.sync.dma_start(
                    AP(ar_in[:], 0, [[160, 128], [129 * 160, 2], [1, 160]]),
                    stgsb[:])
                nc.sync.dma_start(ar_in[0, 128, 0:NCLS], zlocb[:])
                nc.gpsimd.collective_compute(
                    "AllReduce", ALU.add,
                    replica_groups=[list(range(N_CORES))],
                    ins=[ar_in.opt()], outs=[ar_out.opt()])
                nc.sync.dma_start(
                    sg[:],
                    AP(ar_out[:], 0, [[160, 128], [129 * 160, 2], [1, 160]]))
                nc.sync.dma_start(
                    zrow[:], AP(ar_out[:], 128 * 160, [[0, 128], [1, NCLS]]))
                nc.vector.reciprocal(rzrow[:], zrow[:])
                nc.vector.tensor_mul(
                    sgf[:].rearrange("p t (c o) -> p t c o", c=NCLS),
                    sg[:].rearrange("p t (c o) -> p t c o", c=NCLS),
                    AP(rzrow[:], 0, [[NCLS, 128], [0, 2], [1, 10], [0, 16]]))
                nc.vector.tensor_mul(sqs[:], sgf[:].rearrange("p t f -> p (t f)"),
                                     sgf[:].rearrange("p t f -> p (t f)"))
                nc.vector.reduce_sum(
                    out=nrm[:],
                    in_=sqs[:].rearrange("p t (c o) -> p t o c", c=NCLS),
                    axis=mybir.AxisListType.X)
                lg1 = rts.tile([128, 2, OCH], F32, tag="lg1", name=f"lg1_{it}")
                nc.scalar.activation(lg1[:], nrm[:], AF.Ln)
                lg2 = rts.tile([128, 2, OCH], F32, tag="lg2", name=f"lg2_{it}")
                nc.scalar.activation(lg2[:], nrm[:], AF.Ln, bias=1.0, scale=1.0)
                fac2 = rts.tile([128, 2, OCH], F32, tag="fc2", name=f"fc2_{it}")
                nc.vector.scalar_tensor_tensor(
                    out=fac2[:], in0=lg1[:], scalar=0.5, in1=lg2[:],
                    op0=ALU.mult, op1=ALU.subtract)
                nc.scalar.activation(fac2[:], fac2[:], AF.Exp)
                nc.vector.tensor_mul(
                    o3[:].rearrange("p t (c o) -> p t c o", c=NCLS),
                    sgf[:].rearrange("p t (c o) -> p t c o", c=NCLS),
                    AP(fac2[:], 0, [[2 * OCH, 128], [OCH, 2], [0, 10], [1, OCH]]))

                # ---- agreement u_v and b update ----
                for (n0, n1) in NCH:
                    for mt, msz in ((0, 128), (1, 32)):
                        pA = psM.tile([128, 512], F32, tag="pA")
                        for bh in range(2):
                            nc.tensor.matmul(
                                pA[:msz, 0:n1 - n0],
                                lhsT=o3[:, bh, mt * 128:mt * 128 + msz],
                                rhs=u3f[:, bh, n0:n1],
                                start=(bh == 0), stop=(bh == 1))
                        e2dst, w3sx = ((e2A, w3sA) if mt == 0 else (e2B, w3sB))
                        nc.vector.tensor_mul(e2dst[:, n0:n1], pA[:msz, 0:n1 - n0],
                                             w3sx[:, n0:n1])
                puv = psU.tile([NCLS, NI], F32, tag="uv")
                for kc, (ssb, e2b) in enumerate(((sselA, e2A), (sselB, e2B))):
                    for (n0, n1) in NCH:
                        nc.tensor.matmul(
                            puv[:, n0:n1], lhsT=ssb[:],
                            rhs=e2b[:, n0:n1],
                            start=(kc == 0), stop=(kc == 1))
                nc.vector.reduce_sum(
                    out=uvf[:],
                    in_=puv[:].rearrange("c (k i) -> c i k", k=8),
                    axis=mybir.AxisListType.X)
                nc.vector.tensor_add(b_sb[:], b_sb[:], uvf[:])
            else:
                # ---- final iteration: ReduceScatter, y stays batch-sharded ----
                for bh in range(2):
                    nc.sync.dma_start(
                        AP(rs_in[:], bh * 4 * (BL + 1) * 160,
                           [[(BL + 1) * 160, 4], [160, 32], [1, 160]]),
                        stgsb[:, bh, :])
                # Z replicated into every segment's last row via ones matmul
                pz = psS.tile([8, NCLS], F32, tag="e8g", bufs=1)
                nc.tensor.matmul(pz[:], lhsT=ones_a[:], rhs=expT[:],
                                 start=True, stop=False)
                nc.tensor.matmul(pz[:], lhsT=ones_b[:], rhs=expT2[:],
                                 start=False, stop=True)
                zrep = rts.tile([8, NCLS], BF16)
                nc.scalar.copy(zrep[:], pz[:])
                nc.sync.dma_start(
                    AP(rs_in[:], BL * 160, [[(BL + 1) * 160, 8], [1, NCLS]]),
                    zrep[:])
                nc.gpsimd.collective_compute(
                    "ReduceScatter", ALU.add,
                    replica_groups=[list(range(N_CORES))],
                    ins=[rs_in.opt()], outs=[rs_out.opt()])
                sgB = rts.tile([BL + 1, 160], BF16)
                nc.sync.dma_start(sgB[:], rs_out[:])
                zrowB = rts.tile([BL, NCLS], BF16)
                nc.sync.dma_start(
                    zrowB[:], AP(rs_out[:], BL * 160, [[0, BL], [1, NCLS]]))
                rzB = rts.tile([BL, NCLS], F32)
                nc.vector.reciprocal(rzB[:], zrowB[:])
                sg32 = rts.tile([BL, 160], F32)
                nc.vector.tensor_mul(
                    sg32[:].rearrange("p (c o) -> p c o", c=NCLS),
                    sgB[0:BL, :].rearrange("p (c o) -> p c o", c=NCLS),
                    AP(rzB[:], 0, [[NCLS, BL], [1, 10], [0, 16]]))
                sq32 = rts.tile([BL, 160], F32)
                nc.vector.tensor_mul(sq32[:], sg32[:], sg32[:])
                nr32 = rts.tile([BL, OCH], F32)
                nc.vector.reduce_sum(
                    out=nr32[:],
                    in_=sq32[:].rearrange("p (c o) -> p o c", c=NCLS),
                    axis=mybir.AxisListType.X)
                la = rts.tile([BL, OCH], F32)
                nc.scalar.activation(la[:], nr32[:], AF.Ln)
                lb = rts.tile([BL, OCH], F32)
                nc.scalar.activation(lb[:], nr32[:], AF.Ln, bias=1.0, scale=1.0)
                fc = rts.tile([BL, OCH], F32)
                nc.vector.scalar_tensor_tensor(
                    out=fc[:], in0=la[:], scalar=0.5, in1=lb[:],
                    op0=ALU.mult, op1=ALU.subtract)
                nc.scalar.activation(fc[:], fc[:], AF.Exp)
                o32 = rts.tile([BL, 160], F32)
                nc.vector.tensor_mul(
                    o32[:].rearrange("p (c o) -> p c o", c=NCLS),
                    sg32[:].rearrange("p (c o) -> p c o", c=NCLS),
                    AP(fc[:], 0, [[OCH, BL], [0, 10], [1, OCH]]))
                nc.sync.dma_start(y[:], o32[:])

    nc.compile()
    return nc


_CACHE = {}


def _get_program():
    if "nc" not in _CACHE:
        _CACHE["nc"] = build_program()
    return _CACHE["nc"]


def _host_inputs(x, conv_w, conv_b, prim_w, prim_b, digit_w):
    import ml_dtypes
    bf = ml_dtypes.bfloat16
    x = np.asarray(x, dtype=np.float32)
    conv_w = np.asarray(conv_w, dtype=np.float32)
    conv_b = np.asarray(conv_b, dtype=np.float32)
    prim_w = np.asarray(prim_w, dtype=np.float32)
    prim_b = np.asarray(prim_b, dtype=np.float32)
    digit_w = np.asarray(digit_w, dtype=np.float32)

    # im2col of x: (B, 1, 28, 28) -> (B, 81, 400) windows
    xi = x.reshape(B, 28, 28)
    s0, s1, s2 = xi.strides
    win = np.lib.stride_tricks.as_strided(
        xi, shape=(B, 9, 9, 20, 20), strides=(s0, s1, s2, s1, s2))
    icold_full = np.ascontiguousarray(
        win.reshape(B, 81, 400).transpose(1, 0, 2)).astype(bf)  # (81, B, 400)

    w1 = np.ascontiguousarray(conv_w.reshape(256, 81).T).astype(bf)
    b1 = np.ascontiguousarray(conv_b.reshape(2, 128).T)
    # permute conv2 output channels: oc=(cap*32+chw) -> q_global so that the
    # AllToAll pack per shard m reads 32 contiguous partitions
    oc = np.arange(256)
    cap_, chw = oc // 32, oc % 32
    qg = (chw // 16) * 128 + (chw // 4 % 4) * 32 + cap_ * 4 + (chw % 4)
    perm_inv = np.argsort(qg)
    w2 = np.ascontiguousarray(
        prim_w.reshape(256, 256, 81).transpose(2, 1, 0)[:, :, perm_inv]).astype(bf)
    b2 = np.ascontiguousarray(prim_b[perm_inv].reshape(2, 128).T)
    ssel = np.zeros((160, NCLS), np.float32)
    for c in range(NCLS):
        for o in range(OCH):
            ssel[c * OCH + o, c] = 1.0 / B
    ssel = ssel.astype(bf)
    capsum = np.zeros((128, 2, 8), np.float32)
    expnd = np.zeros((8, 2, 128), np.float32)
    for p in range(128):
        for oc_t in range(2):
            c_of_p = (p % 32) // 4
            capsum[p, oc_t, c_of_p] = 1.0
            expnd[c_of_p, oc_t, p] = 1.0
    capsum = capsum.astype(bf)
    expnd = expnd.astype(bf)
    rexpa = np.zeros((128, 9, 128), np.float32)
    rexpb = np.zeros((16, 9, 128), np.float32)
    for t in range(9):
        for p in range(128):
            i_l = (t * 128 + p) % ISH
            if i_l < 128:
                rexpa[i_l, t, p] = 1.0
            else:
                rexpb[i_l - 128, t, p] = 1.0
    rexpa = rexpa.astype(bf)
    rexpb = rexpb.astype(bf)

    in_maps = []
    for m in range(N_CORES):
        dw = digit_w[m * ISH:(m + 1) * ISH]              # (144, 10, 16, 8)
        w2s_h = np.ascontiguousarray(
            dw.transpose(3, 0, 1, 2).reshape(NI, 160)).astype(bf)
        w3s_h = np.ascontiguousarray(
            dw.transpose(1, 2, 3, 0).reshape(160, NI)).astype(bf)
        in_maps.append({
            "icold": np.ascontiguousarray(icold_full[:, m * BL:(m + 1) * BL, :]),
            "w1": w1, "b1": b1, "w2": w2, "b2": b2,
            "w2s": w2s_h, "w3s": w3s_h, "ssel": ssel,
            "capsum": capsum, "expnd": expnd, "rexpa": rexpa, "rexpb": rexpb,
        })
    return in_maps


def kernel(x, conv_w, conv_b, prim_w, prim_b, digit_w, trace=False):
    nc = _get_program()
    in_maps = _host_inputs(x, conv_w, conv_b, prim_w, prim_b, digit_w)
    res = run_bass_kernel_spmd(nc, in_maps, list(range(N_CORES)), trace=trace)
    out = np.concatenate(
        [res.results[m]["y"] for m in range(N_CORES)], axis=0)
    out = out.reshape(B, NCLS, OCH, 1).astype(np.float32)
    if trace:
        return out, res
    return out
